# revision 10
# baseline (speedup 1.0000x reference)
"""Two-layer GAT (PyG GATConv x2) on 8 Trainium2 NeuronCores via Bass.

Strategy (dst-sharded, graph-parallel):
- Nodes sharded 8 ways by destination range (6250/core, padded to 6272).
- Per layer: local feature matmul -> build a gather table row per node
  [G-per-head|1.0 ... | al_src | al_dst] in bf16 -> on-device AllGather ->
  edge phase: edges sorted by dst window (128 dsts), bulk dma_gather of
  source rows, softmax WITHOUT max-subtraction (exponents bounded, fp32-safe),
  segment-sums via one-hot "staircase" mask matmuls accumulating in PSUM.
  The softmax denominator rides in the same matmul through baked 1.0 columns.
  al_dst is broadcast dst->edges with a matmul whose lhsT is the TRANSPOSED
  mask, generated directly by a second is_equal against a partition-broadcast
  (stride-0 DMA) copy of the per-edge dst-lane values -- no PE transposes.
- leaky_relu inside exp via scalar-engine Lrelu followed by Exp.
"""
import math
import sys

import numpy as np
import ml_dtypes

sys.path.insert(0, '/opt/trn_rl_repo')

bf16 = ml_dtypes.bfloat16

P = 128
NCORE = 8
N = 50000
NSH = 6250
NSHP = 6272          # 49 * 128
NW = NSHP // P       # 49 windows
HALF = 4 * NSHP      # 25088 rows per half-table
CIN = 256
H = 4
D1 = 64
D2 = 32
R1 = 384                  # table-1 row stride (256B-mult; content 272)
R2 = 256                  # table-2 row stride (256B-mult; content 144)
ES1 = 384                 # gather elem count L1 (768B; 256B-mult >= 272)
ES2 = 256                 # gather elem count L2 (512B; 256B-mult >= 144)
NBMAX = 8                 # blocks per gather call
TROWS = 2 * HALF + 256    # table alloc rows (incl. spill pad)
PADREL = 200.0            # dstrel sentinel for pad edges (kills mask column)
NEG = 0.2


def _prep(edge_index):
    """Host-side: shard + sort edges, build schedule and index arrays."""
    src = np.concatenate([edge_index[0], np.arange(N, dtype=np.int64)]).astype(np.int64)
    dst = np.concatenate([edge_index[1], np.arange(N, dtype=np.int64)]).astype(np.int64)
    owner = dst // NSH
    dloc = (dst - owner * NSH).astype(np.int32)
    srcpad = ((src // NSH) * NSHP + (src % NSH)).astype(np.int32)
    w = dloc // P
    drel = (dloc % P).astype(np.int32)
    half = (srcpad >= HALF).astype(np.int32)
    srcrel = np.where(half == 1, srcpad - HALF, srcpad).astype(np.int32)

    # per (core, window, half) edge lists
    counts = np.zeros((NCORE, NW, 2), np.int64)
    percore = []
    for k in range(NCORE):
        sel = np.nonzero(owner == k)[0]
        key = (w[sel] * 2 + half[sel]).astype(np.int64)
        order = np.argsort(key, kind='stable')
        sel = sel[order]
        kk = key[order]
        cnt = np.bincount(kk, minlength=NW * 2).reshape(NW, 2)
        counts[k] = cnt
        percore.append((srcrel[sel], drel[sel], cnt))

    nblk = np.maximum(1, np.ceil(counts.max(axis=0) / P).astype(np.int64))  # [NW,2]
    block_meta = []           # (window, half) per block
    for wi in range(NW):
        for h in range(2):
            block_meta += [(wi, h)] * int(nblk[wi, h])
    totblk = len(block_meta)

    # calls: runs of consecutive same-half blocks, <= NBMAX blocks each
    calls = []                # (b0, nb, half, col0)
    col = 0
    b = 0
    while b < totblk:
        h = block_meta[b][1]
        nb = 1
        while (b + nb < totblk and block_meta[b + nb][1] == h
               and nb < NBMAX):
            nb += 1
        calls.append((b, nb, h, col))
        col += nb * 8
        b += nb
    ccols = col

    # per-core lane arrays
    srcidx = np.zeros((NCORE, totblk, P), np.int16)
    dstrel = np.full((NCORE, totblk, P), PADREL, np.float32)
    for k in range(NCORE):
        es, ed, cnt = percore[k]
        pos = 0
        blk = 0
        for wi in range(NW):
            for h in range(2):
                c = int(cnt[wi, h])
                nb = int(nblk[wi, h])
                lanes = np.arange(c)
                srcidx[k, blk + lanes // P, lanes % P] = es[pos:pos + c]
                dstrel[k, blk + lanes // P, lanes % P] = ed[pos:pos + c]
                pos += c
                blk += nb
        assert pos == len(es)

    # pack call indices: [128, ccols] int16 per core
    srcpk = np.zeros((NCORE, P, ccols), np.int16)
    for k in range(NCORE):
        for (b0, nb, h, c0) in calls:
            ni = nb * P
            flat = srcidx[k, b0:b0 + nb].reshape(ni)   # flat[j*128+p]
            wrap = flat.reshape(-1, 16).T              # [16, ni/16]
            srcpk[k, :, c0:c0 + ni // 16] = np.tile(wrap, (8, 1))
    # host-precomputed one-hot masks, both layouts, streamed from DRAM
    qrange = np.arange(P, dtype=np.float32)
    mt_host = np.zeros((NCORE, P, totblk * P), bf16)
    mtT_host = np.zeros((NCORE, P, totblk * P), bf16)
    for k in range(NCORE):
        oh = (dstrel[k][:, :, None] == qrange[None, None, :])  # [b, lane, q]
        mt_host[k] = np.ascontiguousarray(
            oh.transpose(1, 0, 2).reshape(P, totblk * P)).astype(bf16)
        mtT_host[k] = np.ascontiguousarray(
            oh.transpose(2, 0, 1).reshape(P, totblk * P)).astype(bf16)

    first_of_w = {}
    last_of_w = {}
    for b, (wi, h) in enumerate(block_meta):
        if wi not in first_of_w:
            first_of_w[wi] = b
        last_of_w[wi] = b
    return dict(block_meta=block_meta, calls=calls, totblk=totblk, ccols=ccols,
                srcpk=srcpk, mt_host=mt_host, mtT_host=mtT_host,
                first_of_w=first_of_w, last_of_w=last_of_w)


def _build(sched):
    import concourse.bass as bass
    import concourse.tile as tile
    from concourse import bacc, mybir, library_config
    from concourse.bass import AP

    dt = mybir.dt
    Alu = mybir.AluOpType
    Act = mybir.ActivationFunctionType

    totblk = sched['totblk']
    ccols = sched['ccols']
    calls = sched['calls']
    block_meta = sched['block_meta']
    first_of_w = sched['first_of_w']
    last_of_w = sched['last_of_w']

    nc = bacc.Bacc("TRN2", target_bir_lowering=False, debug=False,
                   num_devices=NCORE, num_swdge_queues=4)

    # ---- I/O ----
    xT = nc.dram_tensor("xT", [CIN, NSHP], dt.bfloat16, kind="ExternalInput")
    W1 = nc.dram_tensor("W1b", [CIN, CIN], dt.bfloat16, kind="ExternalInput")
    W2 = nc.dram_tensor("W2b", [CIN, H * D2], dt.bfloat16, kind="ExternalInput")
    a1r = nc.dram_tensor("a1r", [P, 2 * CIN], dt.float32, kind="ExternalInput")
    a2r = nc.dram_tensor("a2r", [P, 2 * H * D2], dt.float32, kind="ExternalInput")
    b1r = nc.dram_tensor("b1r", [P, CIN], dt.float32, kind="ExternalInput")
    b2r = nc.dram_tensor("b2r", [P, D2], dt.float32, kind="ExternalInput")
    ident_in = nc.dram_tensor("ident_in", [P, P], dt.bfloat16, kind="ExternalInput")
    sidx = nc.dram_tensor("sidx", [P, ccols], dt.int16, kind="ExternalInput")
    mtin = nc.dram_tensor("mtin", [P, totblk * P], dt.bfloat16,
                          kind="ExternalInput")
    mtTin = nc.dram_tensor("mtTin", [P, totblk * P], dt.bfloat16,
                           kind="ExternalInput")
    out_t = nc.dram_tensor("out", [NSHP, D2], dt.float32, kind="ExternalOutput")

    # ---- internal DRAM ----
    t1own = nc.dram_tensor("t1own", [NSHP, R1], dt.bfloat16)
    t2own = nc.dram_tensor("t2own", [NSHP, R2], dt.bfloat16)
    T1 = nc.dram_tensor("T1", [TROWS, R1], dt.bfloat16, addr_space="Shared")
    T2 = nc.dram_tensor("T2", [TROWS, R2], dt.bfloat16, addr_space="Shared")

    rg = [list(range(NCORE))]

    with tile.TileContext(nc) as tc:
        import contextlib
        ctx = contextlib.ExitStack()
        with ctx:
            cpool = ctx.enter_context(tc.tile_pool(name="consts", bufs=1))
            gpool = ctx.enter_context(tc.tile_pool(name="g", bufs=4))
            gspool = ctx.enter_context(tc.tile_pool(name="gs", bufs=3))
            mpool = ctx.enter_context(tc.tile_pool(name="mask", bufs=3))
            tpool = ctx.enter_context(tc.tile_pool(name="maskT", bufs=3))
            spool = ctx.enter_context(tc.tile_pool(name="ssb", bufs=3))
            epool = ctx.enter_context(tc.tile_pool(name="ex", bufs=8))
            wpool = ctx.enter_context(tc.tile_pool(name="wend", bufs=4))
            rowpool = ctx.enter_context(tc.tile_pool(name="trow", bufs=4))
            xpool = ctx.enter_context(tc.tile_pool(name="xt", bufs=4))
            pw_pool = ctx.enter_context(tc.tile_pool(name="pw", bufs=2, space="PSUM"))
            tps_pool = ctx.enter_context(tc.tile_pool(name="tps", bufs=2, space="PSUM"))
            ad_pool = ctx.enter_context(tc.tile_pool(name="adp", bufs=2, space="PSUM"))
            hp_pool = ctx.enter_context(tc.tile_pool(name="hps", bufs=2, space="PSUM"))

            nc.gpsimd.load_library(library_config.mlp)

            # ---- persistent constants ----
            ident_sb = cpool.tile([P, P], dt.bfloat16, tag="ident")
            nc.sync.dma_start(out=ident_sb[:], in_=ident_in[:, :])
            w1a = cpool.tile([P, CIN], dt.bfloat16, tag="w1a")
            nc.sync.dma_start(out=w1a[:], in_=W1[0:P, :])
            w1b = cpool.tile([P, CIN], dt.bfloat16, tag="w1b")
            nc.sync.dma_start(out=w1b[:], in_=W1[P:2 * P, :])
            w2a = cpool.tile([P, H * D2], dt.bfloat16, tag="w2a")
            nc.sync.dma_start(out=w2a[:], in_=W2[0:P, :])
            w2b = cpool.tile([P, H * D2], dt.bfloat16, tag="w2b")
            nc.sync.dma_start(out=w2b[:], in_=W2[P:2 * P, :])
            a1_sb = cpool.tile([P, 2 * CIN], dt.float32, tag="a1")
            nc.sync.dma_start(out=a1_sb[:], in_=a1r[:, :])
            a2_sb = cpool.tile([P, 2 * H * D2], dt.float32, tag="a2")
            nc.sync.dma_start(out=a2_sb[:], in_=a2r[:, :])
            b1_sb = cpool.tile([P, CIN], dt.float32, tag="b1")
            nc.sync.dma_start(out=b1_sb[:], in_=b1r[:, :])
            b2_sb = cpool.tile([P, D2], dt.float32, tag="b2")
            nc.sync.dma_start(out=b2_sb[:], in_=b2r[:, :])
            sidx_sb = cpool.tile([P, ccols], dt.int16, tag="sidx")
            nc.sync.dma_start(out=sidx_sb[:], in_=sidx[:, :])
            alown1 = cpool.tile([P, NW * 2 * H], dt.bfloat16, tag="alo1")
            alown2 = cpool.tile([P, NW * 2 * H], dt.bfloat16, tag="alo2")
            hp_sb = cpool.tile([P, NW * CIN], dt.bfloat16, tag="hp")

            def node_l1(wi):
                """x@W1 for window wi; build table row; als."""
                ps = hp_pool.tile([P, CIN], dt.float32, space="PSUM", tag="hps")
                la = xpool.tile([P, P], dt.bfloat16, tag="xa")
                nc.sync.dma_start(out=la[:], in_=xT[0:P, wi * P:(wi + 1) * P])
                lb = xpool.tile([P, P], dt.bfloat16, tag="xb")
                nc.sync.dma_start(out=lb[:], in_=xT[P:2 * P, wi * P:(wi + 1) * P])
                nc.tensor.matmul(out=ps[:], lhsT=la[:], rhs=w1a[:],
                                 start=True, stop=False)
                nc.tensor.matmul(out=ps[:], lhsT=lb[:], rhs=w1b[:],
                                 start=False, stop=True)
                node_tail(wi, ps, CIN, D1, R1, alown1, a1_sb, t1own)

            def node_l2(wi):
                """h'@W2 for window wi (h' from hp_sb); table row; als."""
                hsl = hp_sb[:, wi * CIN:(wi + 1) * CIN]
                ps = hp_pool.tile([P, H * D2], dt.float32, space="PSUM", tag="hps")
                for kc in range(2):
                    tp = tps_pool.tile([P, P], dt.bfloat16, space="PSUM", tag="tps")
                    nc.tensor.transpose(out=tp[:], in_=hsl[:, kc * P:(kc + 1) * P],
                                        identity=ident_sb[:])
                    hT = spool.tile([P, P], dt.bfloat16, tag="hT")
                    nc.scalar.copy(out=hT[:], in_=tp[:])
                    nc.tensor.matmul(out=ps[:], lhsT=hT[:],
                                     rhs=(w2a if kc == 0 else w2b)[:],
                                     start=(kc == 0), stop=(kc == 1))
                node_tail(wi, ps, H * D2, D2, R2, alown2, a2_sb, t2own)

            def node_tail(wi, ps, width, dg, rw, alo, a_sb, town):
                # als: tmp = ps(x2) * a_rep ; reduce per head
                tmp = wpool.tile([P, 2 * width], dt.float32, tag="altmp")
                nc.vector.tensor_tensor(
                    out=tmp[:], in0=ps[:].unsqueeze(1).to_broadcast([P, 2, width]),
                    in1=a_sb[:].rearrange("p (t x) -> p t x", t=2),
                    op=Alu.mult)
                alof = wpool.tile([P, 8], dt.float32, tag="alof")
                nc.vector.tensor_reduce(
                    out=alof[:],
                    in_=tmp[:].rearrange("p (t h d) -> p t h d", t=2, h=H),
                    axis=mybir.AxisListType.X, op=Alu.add)
                nc.scalar.copy(out=alo[:, wi * 8:wi * 8 + 8], in_=alof[:])

                # table row
                tr = rowpool.tile([P, rw], dt.bfloat16, tag="trow")
                nc.gpsimd.memset(tr[:], 1.0)
                nc.scalar.copy(
                    out=tr[:, 0:H * (dg + 2)].rearrange(
                        "p (h y) -> p h y", h=H)[:, :, 0:dg],
                    in_=ps[:].rearrange("p (h d) -> p h d", h=H))
                nc.scalar.copy(
                    out=tr[:, H * (dg + 2):H * (dg + 2) + 8], in_=alof[:])
                nc.sync.dma_start(out=town[wi * P:(wi + 1) * P, :], in_=tr[:])

            def edge_phase(layer):
                T = T1 if layer == 1 else T2
                rw = R1 if layer == 1 else R2
                es = ES1 if layer == 1 else ES2
                dg = D1 if layer == 1 else D2
                width = H * (dg + 2)
                alo = alown1 if layer == 1 else alown2
                pw = {}
                qn = 0
                for ci, (b0, nb, hf, c0) in enumerate(calls):
                    ni = nb * P
                    g = gpool.tile([P, NBMAX * es], dt.bfloat16, tag="g")
                    in_ap = AP(T, hf * HALF * rw, [[rw, TROWS - hf * HALF], [1, es]])
                    nc.gpsimd.dma_gather(
                        g[:, 0:nb * es].rearrange("p (b w) -> p b w", w=es),
                        in_ap, sidx_sb[:, c0:c0 + ni // 16], ni, ni, es,
                        elem_step=rw, queue_num=qn)
                    qn = (qn + 1) % 4

                    # masks streamed from DRAM (host-precomputed one-hots)
                    mt = mpool.tile([P, NBMAX * P], dt.bfloat16, tag="m")
                    nc.sync.dma_start(out=mt[:, 0:ni],
                                      in_=mtin[:, b0 * P:b0 * P + ni])
                    mtT = tpool.tile([P, NBMAX * P], dt.bfloat16, tag="mT")
                    nc.sync.dma_start(out=mtT[:, 0:ni],
                                      in_=mtTin[:, b0 * P:b0 * P + ni])

                    # Ad broadcast dst->edges: one small matmul per block
                    adps = ad_pool.tile([P, NBMAX * H], dt.float32, space="PSUM",
                                        tag="adp")
                    for j in range(nb):
                        b = b0 + j
                        wi = block_meta[b][0]
                        nc.tensor.matmul(
                            out=adps[:, j * H:(j + 1) * H],
                            lhsT=mtT[:, j * P:(j + 1) * P],
                            rhs=alo[:, wi * 8 + 4:wi * 8 + 8],
                            start=True, stop=True)

                    # ex = exp(leaky(al_src + al_dst)) on the scalar engine
                    t = epool.tile([P, NBMAX * H], dt.float32, tag="t")
                    nc.vector.tensor_tensor(
                        out=t[:, 0:nb * H],
                        in0=g[:, 0:nb * es].rearrange(
                            "p (b w) -> p b w", w=es)[:, :, width:width + H],
                        in1=adps[:, 0:nb * H].rearrange("p (b h) -> p b h", h=H),
                        op=Alu.add)
                    u = epool.tile([P, NBMAX * H], dt.float32, tag="u")
                    nc.scalar.activation(out=u[:, 0:nb * H], in_=t[:, 0:nb * H],
                                         func=Act.Prelu, alpha=NEG)
                    exbf = epool.tile([P, NBMAX * H], dt.bfloat16, tag="exbf")
                    nc.scalar.activation(out=exbf[:, 0:nb * H], in_=u[:, 0:nb * H],
                                         func=Act.Exp)

                    # scale: Gs = G(+ones) * ex
                    gs = gspool.tile([P, NBMAX * width], dt.bfloat16, tag="gs")
                    nc.vector.tensor_tensor(
                        out=gs[:, 0:nb * width].rearrange(
                            "p (b h y) -> p b h y", h=H, y=dg + 2),
                        in0=g[:, 0:nb * es].rearrange(
                            "p (b w) -> p b w", w=es)[:, :, 0:width].rearrange(
                            "p b (h y) -> p b h y", y=dg + 2),
                        in1=exbf[:, 0:nb * H].rearrange(
                            "p (b h) -> p b h", h=H).unsqueeze(3).to_broadcast(
                            [P, nb, H, dg + 2]),
                        op=Alu.mult)

                    # matmuls
                    for j in range(nb):
                        b = b0 + j
                        wi = block_meta[b][0]
                        if wi not in pw:
                            pw[wi] = pw_pool.tile([P, width], dt.float32,
                                                  space="PSUM", tag="pw",
                                                  name=f"pw{layer}_{wi}")
                        nc.tensor.matmul(
                            out=pw[wi][:], lhsT=mt[:, j * P:(j + 1) * P],
                            rhs=gs[:, j * width:(j + 1) * width],
                            start=(b == first_of_w[wi]), stop=(b == last_of_w[wi]))
                        if b == last_of_w[wi]:
                            window_end(layer, wi, pw.pop(wi), dg, width)

            def window_end(layer, wi, ps, dg, width):
                if layer == 1:
                    dn = wpool.tile([P, H], dt.float32, tag="dn")
                    nc.vector.tensor_scalar(
                        out=dn[:],
                        in0=ps[:].rearrange("p (h y) -> p h y", y=dg + 2)[:, :, dg:dg + 1],
                        scalar1=1e-30, scalar2=None, op0=Alu.max)
                    rc = wpool.tile([P, H], dt.float32, tag="rc")
                    nc.vector.reciprocal(out=rc[:], in_=dn[:])
                    hr = wpool.tile([P, CIN], dt.float32, tag="hr")
                    nc.vector.tensor_tensor(
                        out=hr[:].rearrange("p (h d) -> p h d", h=H),
                        in0=ps[:].rearrange("p (h y) -> p h y", y=dg + 2)[:, :, 0:dg],
                        in1=rc[:].unsqueeze(2).to_broadcast([P, H, dg]),
                        op=Alu.mult)
                    nc.vector.tensor_tensor(out=hr[:], in0=hr[:], in1=b1_sb[:],
                                            op=Alu.add)
                    # leaky_relu on the scalar engine, straight into hp_sb
                    nc.scalar.activation(
                        out=hp_sb[:, wi * CIN:(wi + 1) * CIN], in_=hr[:],
                        func=mybir.ActivationFunctionType.Prelu, alpha=NEG)
                    node_l2(wi)
                else:
                    dn = wpool.tile([P, H], dt.float32, tag="dn")
                    nc.vector.tensor_scalar(
                        out=dn[:],
                        in0=ps[:].rearrange("p (h y) -> p h y", y=dg + 2)[:, :, dg:dg + 1],
                        scalar1=1e-30, scalar2=4.0, op0=Alu.max,
                        op1=Alu.mult)
                    rc = wpool.tile([P, H], dt.float32, tag="rc")
                    nc.vector.reciprocal(out=rc[:], in_=dn[:])
                    tmp = wpool.tile([P, H * D2], dt.float32, tag="otmp")
                    nc.vector.tensor_tensor(
                        out=tmp[:].rearrange("p (h d) -> p h d", h=H),
                        in0=ps[:].rearrange("p (h y) -> p h y", y=dg + 2)[:, :, 0:dg],
                        in1=rc[:].unsqueeze(2).to_broadcast([P, H, D2]),
                        op=Alu.mult)
                    red = wpool.tile([P, D2], dt.float32, tag="red")
                    nc.vector.tensor_reduce(
                        out=red[:],
                        in_=tmp[:].rearrange("p (h d) -> p d h", h=H),
                        axis=mybir.AxisListType.X, op=Alu.add)
                    nc.vector.tensor_tensor(out=red[:], in0=red[:], in1=b2_sb[:],
                                            op=Alu.add)
                    nc.sync.dma_start(out=out_t[wi * P:(wi + 1) * P, :], in_=red[:])

            # ---------- program ----------
            for wi in range(NW):
                node_l1(wi)
            nc.gpsimd.collective_compute(
                "AllGather", mybir.AluOpType.bypass, replica_groups=rg,
                ins=[t1own[:, :]], outs=[T1[0:NCORE * NSHP, :]])
            edge_phase(1)            # fuses node_l2 per finished window
            nc.gpsimd.collective_compute(
                "AllGather", mybir.AluOpType.bypass, replica_groups=rg,
                ins=[t2own[:, :]], outs=[T2[0:NCORE * NSHP, :]])
            edge_phase(2)

    nc.compile()
    return nc


def _host_inputs(inputs, sched):
    x = np.asarray(inputs['x'], np.float32)
    W1 = np.asarray(inputs['W1'], np.float32)
    W2 = np.asarray(inputs['W2'], np.float32)
    a_src1 = np.asarray(inputs['a_src1'], np.float32)
    a_dst1 = np.asarray(inputs['a_dst1'], np.float32)
    a_src2 = np.asarray(inputs['a_src2'], np.float32)
    a_dst2 = np.asarray(inputs['a_dst2'], np.float32)
    b1 = np.asarray(inputs['b1'], np.float32)
    b2 = np.asarray(inputs['b2'], np.float32)

    a1r = np.tile(np.concatenate([a_src1.reshape(-1), a_dst1.reshape(-1)])[None, :],
                  (P, 1)).astype(np.float32)
    a2r = np.tile(np.concatenate([a_src2.reshape(-1), a_dst2.reshape(-1)])[None, :],
                  (P, 1)).astype(np.float32)
    b1r = np.tile(b1[None, :], (P, 1)).astype(np.float32)
    b2r = np.tile(b2[None, :], (P, 1)).astype(np.float32)
    ident = np.eye(P, dtype=np.float32).astype(bf16)

    in_maps = []
    for k in range(NCORE):
        xk = np.zeros((NSHP, CIN), np.float32)
        xk[:NSH] = x[k * NSH:(k + 1) * NSH]
        in_maps.append({
            "xT": np.ascontiguousarray(xk.T).astype(bf16),
            "W1b": W1.astype(bf16),
            "W2b": W2.astype(bf16),
            "a1r": a1r, "a2r": a2r, "b1r": b1r, "b2r": b2r,
            "ident_in": ident,
            "sidx": sched['srcpk'][k],
            "mtin": sched['mt_host'][k],
            "mtTin": sched['mtT_host'][k],
        })
    return in_maps


def kernel(**inputs):
    import os
    from concourse.bass_utils import run_bass_kernel_spmd

    edge_index = np.asarray(inputs['edge_index'])
    sched = _prep(edge_index)
    nc = _build(sched)
    in_maps = _host_inputs(inputs, sched)

    trace = os.environ.get("KERNEL_TRACE") == "1"
    if trace:
        try:
            import profhook
            profhook.install()
        except ImportError:
            pass
    res = run_bass_kernel_spmd(nc, in_maps, core_ids=list(range(NCORE)),
                               trace=trace)
    if trace and res.exec_time_ns:
        print(f"HW exec time: {res.exec_time_ns} ns")
        kernel.exec_time_ns = res.exec_time_ns
        kernel.res = res

    out = np.zeros((N, D2), np.float32)
    for k in range(NCORE):
        out[k * NSH:(k + 1) * NSH] = res.results[k]["out"][:NSH]
    return out


# revision 11
# speedup vs baseline: 1.1334x; 1.1334x over previous
"""Two-layer GAT (PyG GATConv x2) on 8 Trainium2 NeuronCores via Bass.

Strategy (dst-sharded, graph-parallel):
- Nodes sharded 8 ways by destination range (6250/core, padded to 6272).
- Per layer: local feature matmul -> build a gather table row per node
  [G-per-head|1.0 ... | al_src | al_dst] in bf16 -> on-device AllGather
  (split into two row-chunks A/B so the collectives overlap compute) ->
  edge phase: edges sorted by dst window (128 dsts), bulk dma_gather of
  source rows (trailing pad lanes skipped via a per-core count register),
  softmax WITHOUT max-subtraction (exponents bounded, fp32-safe),
  segment-sums via one-hot "staircase" mask matmuls accumulating in PSUM.
  The softmax denominator rides in the same matmul through baked 1.0 columns.
  Both mask layouts (lhsT for the segment-sum and its transpose for the
  al_dst dst->edge broadcast) are host-precomputed and streamed from DRAM.
- leaky_relu via scalar-engine Prelu (exact alpha=0.2); exp on scalar.
- Layer-2 node matmul fused into layer-1 window_end; AG2A fires mid-edge1.
"""
import math
import sys

import numpy as np
import ml_dtypes

sys.path.insert(0, '/opt/trn_rl_repo')

bf16 = ml_dtypes.bfloat16

P = 128
NCORE = 8
N = 50000
NSH = 6250
NSHP = 6272          # 49 * 128
NW = NSHP // P       # 49 windows
SPLIT = 3200         # local rows < SPLIT -> table A (windows 0-24)
NWA = SPLIT // P     # 25 windows in chunk A
ROWS_A = NCORE * SPLIT            # 25600 (< int16 max)
ROWS_B = NCORE * (NSHP - SPLIT)   # 24576
CIN = 256
H = 4
D1 = 64
D2 = 32
R1 = 384                  # table-1 row stride (256B-mult; content 272)
R2 = 256                  # table-2 row stride (256B-mult; content 144)
ES1 = 384                 # gather elem count L1 (768B)
ES2 = 256                 # gather elem count L2 (512B)
NBMAX = 8                 # blocks per gather call (1024-desc SWDGE ring limit)
PADREL = 200.0            # dstrel sentinel for pad edges (kills mask column)
NEG = 0.2


def _prep(edge_index):
    """Host-side: shard + sort edges, build schedule and index arrays."""
    src = np.concatenate([edge_index[0], np.arange(N, dtype=np.int64)]).astype(np.int64)
    dst = np.concatenate([edge_index[1], np.arange(N, dtype=np.int64)]).astype(np.int64)
    owner = dst // NSH
    dloc = (dst - owner * NSH).astype(np.int32)
    w = dloc // P
    drel = (dloc % P).astype(np.int32)
    owner_s = (src // NSH).astype(np.int32)
    locpad = (src % NSH).astype(np.int32)
    half = (locpad >= SPLIT).astype(np.int32)
    srcrel = np.where(half == 1,
                      owner_s * (NSHP - SPLIT) + (locpad - SPLIT),
                      owner_s * SPLIT + locpad).astype(np.int32)

    # per (core, window, half) edge lists
    counts = np.zeros((NCORE, NW, 2), np.int64)
    percore = []
    for k in range(NCORE):
        sel = np.nonzero(owner == k)[0]
        key = (w[sel] * 2 + half[sel]).astype(np.int64)
        order = np.argsort(key, kind='stable')
        sel = sel[order]
        kk = key[order]
        cnt = np.bincount(kk, minlength=NW * 2).reshape(NW, 2)
        counts[k] = cnt
        percore.append((srcrel[sel], drel[sel], cnt, sel))

    nblk = np.maximum(1, np.ceil(counts.max(axis=0) / P).astype(np.int64))  # [NW,2]
    block_meta = []           # (window, half) per block
    for wi in range(NW):
        for h in range(2):
            block_meta += [(wi, h)] * int(nblk[wi, h])
    totblk = len(block_meta)

    # calls: runs of consecutive same-half blocks, <= NBMAX blocks each
    # (calls never span (window, half) groups: half flips between groups)
    calls = []                # (b0, nb, half, col0)
    col = 0
    b = 0
    while b < totblk:
        h = block_meta[b][1]
        nb = 1
        while (b + nb < totblk and block_meta[b + nb][1] == h
               and nb < NBMAX):
            nb += 1
        calls.append((b, nb, h, col))
        col += nb * 8
        b += nb
    ccols = col

    # per-core lane arrays; pads = -1 (skipped by the count register)
    srcidx = np.full((NCORE, totblk, P), -1, np.int16)
    dstrel = np.full((NCORE, totblk, P), PADREL, np.float32)
    for k in range(NCORE):
        es, ed, cnt, _ = percore[k]
        pos = 0
        blk = 0
        for wi in range(NW):
            for h in range(2):
                c = int(cnt[wi, h])
                nb = int(nblk[wi, h])
                lanes = np.arange(c)
                srcidx[k, blk + lanes // P, lanes % P] = es[pos:pos + c]
                dstrel[k, blk + lanes // P, lanes % P] = ed[pos:pos + c]
                pos += c
                blk += nb
        assert pos == len(es)

    # per-call valid counts; guarantee >=1 valid index per call
    cnts = np.zeros((NCORE, len(calls)), np.int32)
    for k in range(NCORE):
        for ci, (b0, nb, h, c0) in enumerate(calls):
            flat = srcidx[k, b0:b0 + nb].reshape(nb * P)
            valid = int((flat >= 0).sum())
            assert (flat[:valid] >= 0).all(), "pads must be trailing"
            if valid == 0:
                srcidx[k, b0, 0] = 0
                valid = 1
            cnts[k, ci] = valid

    # pack call indices: [128, ccols] int16 per core
    srcpk = np.zeros((NCORE, P, ccols), np.int16)
    for k in range(NCORE):
        for (b0, nb, h, c0) in calls:
            ni = nb * P
            flat = srcidx[k, b0:b0 + nb].reshape(ni)   # flat[j*128+p]
            wrap = flat.reshape(-1, 16).T              # [16, ni/16]
            srcpk[k, :, c0:c0 + ni // 16] = np.tile(wrap, (8, 1))

    # host-precomputed one-hot masks, both layouts, streamed from DRAM
    qrange = np.arange(P, dtype=np.float32)
    mt_host = np.zeros((NCORE, P, totblk * P), bf16)
    mtT_host = np.zeros((NCORE, P, totblk * P), bf16)
    for k in range(NCORE):
        oh = (dstrel[k][:, :, None] == qrange[None, None, :])  # [b, lane, q]
        mt_host[k] = np.ascontiguousarray(
            oh.transpose(1, 0, 2).reshape(P, totblk * P)).astype(bf16)
        mtT_host[k] = np.ascontiguousarray(
            oh.transpose(2, 0, 1).reshape(P, totblk * P)).astype(bf16)

    # sanity: reconstruct (src, dst) pairs from the schedule for each core
    bm = np.array(block_meta)
    for k in range(NCORE):
        dr = dstrel[k]
        real = (dr != PADREL)
        wins = bm[:, 0][:, None] * P
        halves = bm[:, 1][:, None]
        rel = srcidx[k].astype(np.int64)
        src_rec = np.where(
            halves == 1,
            (rel // (NSHP - SPLIT)) * NSH + SPLIT + rel % (NSHP - SPLIT),
            (rel // SPLIT) * NSH + rel % SPLIT)
        dst_rec = k * NSH + wins + dr.astype(np.int64)
        got = np.stack([src_rec[real], dst_rec[real]], 1)
        sel = percore[k][3]
        want = np.stack([src[sel], dst[sel]], 1)
        gs_ = got[np.lexsort(got.T)]
        ws_ = want[np.lexsort(want.T)]
        assert gs_.shape == ws_.shape and (gs_ == ws_).all(), \
            f"schedule mismatch core {k}"

    first_of_w = {}
    last_of_w = {}
    for b, (wi, h) in enumerate(block_meta):
        if wi not in first_of_w:
            first_of_w[wi] = b
        last_of_w[wi] = b
    return dict(block_meta=block_meta, calls=calls, totblk=totblk, ccols=ccols,
                srcpk=srcpk, mt_host=mt_host, mtT_host=mtT_host, cnts=cnts,
                first_of_w=first_of_w, last_of_w=last_of_w)


def _build(sched):
    import concourse.bass as bass
    import concourse.tile as tile
    from concourse import bacc, mybir, library_config
    from concourse.bass import AP

    dt = mybir.dt
    Alu = mybir.AluOpType
    Act = mybir.ActivationFunctionType

    totblk = sched['totblk']
    ccols = sched['ccols']
    calls = sched['calls']
    ncalls = len(calls)
    block_meta = sched['block_meta']
    first_of_w = sched['first_of_w']
    last_of_w = sched['last_of_w']

    nc = bacc.Bacc("TRN2", target_bir_lowering=False, debug=False,
                   num_devices=NCORE, num_swdge_queues=4)

    # ---- I/O ----
    xT = nc.dram_tensor("xT", [CIN, NSHP], dt.bfloat16, kind="ExternalInput")
    W1 = nc.dram_tensor("W1b", [CIN, CIN], dt.bfloat16, kind="ExternalInput")
    W2 = nc.dram_tensor("W2b", [CIN, H * D2], dt.bfloat16, kind="ExternalInput")
    a1r = nc.dram_tensor("a1r", [P, 2 * CIN], dt.float32, kind="ExternalInput")
    a2r = nc.dram_tensor("a2r", [P, 2 * H * D2], dt.float32, kind="ExternalInput")
    b1r = nc.dram_tensor("b1r", [P, CIN], dt.float32, kind="ExternalInput")
    b2r = nc.dram_tensor("b2r", [P, D2], dt.float32, kind="ExternalInput")
    ident_in = nc.dram_tensor("ident_in", [P, P], dt.bfloat16, kind="ExternalInput")
    sidx = nc.dram_tensor("sidx", [P, ccols], dt.int16, kind="ExternalInput")
    cnt_in = nc.dram_tensor("cnt_in", [1, ncalls], dt.int32, kind="ExternalInput")
    mtin = nc.dram_tensor("mtin", [P, totblk * P], dt.bfloat16,
                          kind="ExternalInput")
    mtTin = nc.dram_tensor("mtTin", [P, totblk * P], dt.bfloat16,
                           kind="ExternalInput")
    out_t = nc.dram_tensor("out", [NSHP, D2], dt.float32, kind="ExternalOutput")

    # ---- internal DRAM (A/B row-chunk split for overlapped AllGathers) ----
    t1ownA = nc.dram_tensor("t1ownA", [SPLIT, R1], dt.bfloat16)
    t1ownB = nc.dram_tensor("t1ownB", [NSHP - SPLIT, R1], dt.bfloat16)
    t2ownA = nc.dram_tensor("t2ownA", [SPLIT, R2], dt.bfloat16)
    t2ownB = nc.dram_tensor("t2ownB", [NSHP - SPLIT, R2], dt.bfloat16)
    T1A = nc.dram_tensor("T1A", [ROWS_A + P, R1], dt.bfloat16, addr_space="Shared")
    T1B = nc.dram_tensor("T1B", [ROWS_B + P, R1], dt.bfloat16, addr_space="Shared")
    T2A = nc.dram_tensor("T2A", [ROWS_A + P, R2], dt.bfloat16, addr_space="Shared")
    T2B = nc.dram_tensor("T2B", [ROWS_B + P, R2], dt.bfloat16, addr_space="Shared")

    rg = [list(range(NCORE))]

    with tile.TileContext(nc) as tc:
        import contextlib
        ctx = contextlib.ExitStack()
        with ctx:
            cpool = ctx.enter_context(tc.tile_pool(name="consts", bufs=1))
            gpool = ctx.enter_context(tc.tile_pool(name="g", bufs=4))
            gspool = ctx.enter_context(tc.tile_pool(name="gs", bufs=3))
            mpool = ctx.enter_context(tc.tile_pool(name="mask", bufs=3))
            tpool = ctx.enter_context(tc.tile_pool(name="maskT", bufs=3))
            spool = ctx.enter_context(tc.tile_pool(name="ssb", bufs=3))
            epool = ctx.enter_context(tc.tile_pool(name="ex", bufs=8))
            wpool = ctx.enter_context(tc.tile_pool(name="wend", bufs=4))
            rowpool = ctx.enter_context(tc.tile_pool(name="trow", bufs=4))
            xpool = ctx.enter_context(tc.tile_pool(name="xt", bufs=4))
            pw_pool = ctx.enter_context(tc.tile_pool(name="pw", bufs=2, space="PSUM"))
            tps_pool = ctx.enter_context(tc.tile_pool(name="tps", bufs=2, space="PSUM"))
            ad_pool = ctx.enter_context(tc.tile_pool(name="adp", bufs=2, space="PSUM"))
            hp_pool = ctx.enter_context(tc.tile_pool(name="hps", bufs=2, space="PSUM"))

            nc.gpsimd.load_library(library_config.mlp)

            # ---- persistent constants ----
            ident_sb = cpool.tile([P, P], dt.bfloat16, tag="ident")
            nc.sync.dma_start(out=ident_sb[:], in_=ident_in[:, :])
            w1a = cpool.tile([P, CIN], dt.bfloat16, tag="w1a")
            nc.sync.dma_start(out=w1a[:], in_=W1[0:P, :])
            w1b = cpool.tile([P, CIN], dt.bfloat16, tag="w1b")
            nc.sync.dma_start(out=w1b[:], in_=W1[P:2 * P, :])
            w2a = cpool.tile([P, H * D2], dt.bfloat16, tag="w2a")
            nc.sync.dma_start(out=w2a[:], in_=W2[0:P, :])
            w2b = cpool.tile([P, H * D2], dt.bfloat16, tag="w2b")
            nc.sync.dma_start(out=w2b[:], in_=W2[P:2 * P, :])
            a1_sb = cpool.tile([P, 2 * CIN], dt.float32, tag="a1")
            nc.sync.dma_start(out=a1_sb[:], in_=a1r[:, :])
            a2_sb = cpool.tile([P, 2 * H * D2], dt.float32, tag="a2")
            nc.sync.dma_start(out=a2_sb[:], in_=a2r[:, :])
            b1_sb = cpool.tile([P, CIN], dt.float32, tag="b1")
            nc.sync.dma_start(out=b1_sb[:], in_=b1r[:, :])
            b2_sb = cpool.tile([P, D2], dt.float32, tag="b2")
            nc.sync.dma_start(out=b2_sb[:], in_=b2r[:, :])
            sidx_sb = cpool.tile([P, ccols], dt.int16, tag="sidx")
            nc.sync.dma_start(out=sidx_sb[:], in_=sidx[:, :])
            cnt_sb = cpool.tile([1, ncalls], dt.int32, tag="cnt")
            nc.sync.dma_start(out=cnt_sb[:], in_=cnt_in[:, :])
            alown1 = cpool.tile([P, NW * 2 * H], dt.bfloat16, tag="alo1")
            alown2 = cpool.tile([P, NW * 2 * H], dt.bfloat16, tag="alo2")
            hp_sb = cpool.tile([P, NW * CIN], dt.bfloat16, tag="hp")

            cnt_reg = nc.gpsimd.alloc_register("gcnt")

            # prime gather buffers so skipped pad lanes hold finite data
            for _ in range(4):
                tt = gpool.tile([P, NBMAX * ES1], dt.bfloat16, tag="g")
                nc.vector.memset(tt[:], 1.0)

            def town_slice(town_a, town_b, wi):
                if wi < NWA:
                    return town_a[wi * P:(wi + 1) * P, :]
                return town_b[(wi - NWA) * P:(wi - NWA + 1) * P, :]

            def node_l1(wi):
                """x@W1 for window wi; build table row; als."""
                ps = hp_pool.tile([P, CIN], dt.float32, space="PSUM", tag="hps")
                la = xpool.tile([P, P], dt.bfloat16, tag="xa")
                nc.sync.dma_start(out=la[:], in_=xT[0:P, wi * P:(wi + 1) * P])
                lb = xpool.tile([P, P], dt.bfloat16, tag="xb")
                nc.sync.dma_start(out=lb[:], in_=xT[P:2 * P, wi * P:(wi + 1) * P])
                nc.tensor.matmul(out=ps[:], lhsT=la[:], rhs=w1a[:],
                                 start=True, stop=False)
                nc.tensor.matmul(out=ps[:], lhsT=lb[:], rhs=w1b[:],
                                 start=False, stop=True)
                node_tail(wi, ps, CIN, D1, alown1, a1_sb,
                          town_slice(t1ownA, t1ownB, wi))

            def node_l2(wi):
                """h'@W2 for window wi (h' from hp_sb); table row; als."""
                hsl = hp_sb[:, wi * CIN:(wi + 1) * CIN]
                ps = hp_pool.tile([P, H * D2], dt.float32, space="PSUM", tag="hps")
                for kc in range(2):
                    tp = tps_pool.tile([P, P], dt.bfloat16, space="PSUM", tag="tps")
                    nc.tensor.transpose(out=tp[:], in_=hsl[:, kc * P:(kc + 1) * P],
                                        identity=ident_sb[:])
                    hT = spool.tile([P, P], dt.bfloat16, tag="hT")
                    nc.scalar.copy(out=hT[:], in_=tp[:])
                    nc.tensor.matmul(out=ps[:], lhsT=hT[:],
                                     rhs=(w2a if kc == 0 else w2b)[:],
                                     start=(kc == 0), stop=(kc == 1))
                node_tail(wi, ps, H * D2, D2, alown2, a2_sb,
                          town_slice(t2ownA, t2ownB, wi))

            def node_tail(wi, ps, width, dg, alo, a_sb, town_rows):
                # als: tmp = ps(x2) * a_rep ; reduce per head
                tmp = wpool.tile([P, 2 * width], dt.float32, tag="altmp")
                nc.vector.tensor_tensor(
                    out=tmp[:], in0=ps[:].unsqueeze(1).to_broadcast([P, 2, width]),
                    in1=a_sb[:].rearrange("p (t x) -> p t x", t=2),
                    op=Alu.mult)
                alof = wpool.tile([P, 8], dt.float32, tag="alof")
                nc.vector.tensor_reduce(
                    out=alof[:],
                    in_=tmp[:].rearrange("p (t h d) -> p t h d", t=2, h=H),
                    axis=mybir.AxisListType.X, op=Alu.add)
                nc.scalar.copy(out=alo[:, wi * 8:wi * 8 + 8], in_=alof[:])

                # table row
                tr = rowpool.tile([P, R1 if width == CIN else R2], dt.bfloat16,
                                  tag="trow")
                nc.gpsimd.memset(tr[:], 1.0)
                nc.scalar.copy(
                    out=tr[:, 0:H * (dg + 2)].rearrange(
                        "p (h y) -> p h y", h=H)[:, :, 0:dg],
                    in_=ps[:].rearrange("p (h d) -> p h d", h=H))
                nc.scalar.copy(
                    out=tr[:, H * (dg + 2):H * (dg + 2) + 8], in_=alof[:])
                nc.sync.dma_start(out=town_rows, in_=tr[:])

            def ag(town, T, rows):
                nc.gpsimd.collective_compute(
                    "AllGather", mybir.AluOpType.bypass, replica_groups=rg,
                    ins=[town[:, :]], outs=[T[0:rows, :]])

            def edge_phase(layer):
                TA = T1A if layer == 1 else T2A
                TB = T1B if layer == 1 else T2B
                rw = R1 if layer == 1 else R2
                es = ES1 if layer == 1 else ES2
                dg = D1 if layer == 1 else D2
                width = H * (dg + 2)
                alo = alown1 if layer == 1 else alown2
                pw = {}
                qn = 0
                for ci, (b0, nb, hf, c0) in enumerate(calls):
                    ni = nb * P
                    g = gpool.tile([P, NBMAX * es], dt.bfloat16, tag="g")
                    T = TA if hf == 0 else TB
                    rows = (ROWS_A if hf == 0 else ROWS_B) + P
                    in_ap = AP(T, 0, [[rw, rows], [1, es]])
                    nc.gpsimd.reg_load(cnt_reg, cnt_sb[0:1, ci:ci + 1])
                    nc.gpsimd.dma_gather(
                        g[:, 0:nb * es].rearrange("p (b w) -> p b w", w=es),
                        in_ap, sidx_sb[:, c0:c0 + ni // 16], ni, cnt_reg, es,
                        elem_step=rw, queue_num=qn)
                    qn = (qn + 1) % 4

                    # masks streamed from DRAM (host-precomputed one-hots)
                    mt = mpool.tile([P, NBMAX * P], dt.bfloat16, tag="m")
                    nc.sync.dma_start(out=mt[:, 0:ni],
                                      in_=mtin[:, b0 * P:b0 * P + ni])
                    mtT = tpool.tile([P, NBMAX * P], dt.bfloat16, tag="mT")
                    nc.scalar.dma_start(out=mtT[:, 0:ni],
                                        in_=mtTin[:, b0 * P:b0 * P + ni])

                    # Ad broadcast dst->edges: one small matmul per block
                    adps = ad_pool.tile([P, NBMAX * H], dt.float32, space="PSUM",
                                        tag="adp")
                    for j in range(nb):
                        b = b0 + j
                        wi = block_meta[b][0]
                        nc.tensor.matmul(
                            out=adps[:, j * H:(j + 1) * H],
                            lhsT=mtT[:, j * P:(j + 1) * P],
                            rhs=alo[:, wi * 8 + 4:wi * 8 + 8],
                            start=True, stop=True)

                    # ex = exp(leaky(al_src + al_dst)) on the scalar engine
                    t = epool.tile([P, NBMAX * H], dt.float32, tag="t")
                    nc.vector.tensor_tensor(
                        out=t[:, 0:nb * H],
                        in0=g[:, 0:nb * es].rearrange(
                            "p (b w) -> p b w", w=es)[:, :, width:width + H],
                        in1=adps[:, 0:nb * H].rearrange("p (b h) -> p b h", h=H),
                        op=Alu.add)
                    u = epool.tile([P, NBMAX * H], dt.float32, tag="u")
                    nc.scalar.activation(out=u[:, 0:nb * H], in_=t[:, 0:nb * H],
                                         func=Act.Prelu, alpha=NEG)
                    exbf = epool.tile([P, NBMAX * H], dt.bfloat16, tag="exbf")
                    nc.scalar.activation(out=exbf[:, 0:nb * H], in_=u[:, 0:nb * H],
                                         func=Act.Exp)

                    # scale: Gs = G(+ones) * ex
                    gs = gspool.tile([P, NBMAX * width], dt.bfloat16, tag="gs")
                    nc.vector.tensor_tensor(
                        out=gs[:, 0:nb * width].rearrange(
                            "p (b h y) -> p b h y", h=H, y=dg + 2),
                        in0=g[:, 0:nb * es].rearrange(
                            "p (b w) -> p b w", w=es)[:, :, 0:width].rearrange(
                            "p b (h y) -> p b h y", y=dg + 2),
                        in1=exbf[:, 0:nb * H].rearrange(
                            "p (b h) -> p b h", h=H).unsqueeze(3).to_broadcast(
                            [P, nb, H, dg + 2]),
                        op=Alu.mult)

                    # matmuls
                    for j in range(nb):
                        b = b0 + j
                        wi = block_meta[b][0]
                        if wi not in pw:
                            pw[wi] = pw_pool.tile([P, width], dt.float32,
                                                  space="PSUM", tag="pw",
                                                  name=f"pw{layer}_{wi}")
                        nc.tensor.matmul(
                            out=pw[wi][:], lhsT=mt[:, j * P:(j + 1) * P],
                            rhs=gs[:, j * width:(j + 1) * width],
                            start=(b == first_of_w[wi]), stop=(b == last_of_w[wi]))
                        if b == last_of_w[wi]:
                            window_end(layer, wi, pw.pop(wi), dg, width)

            def window_end(layer, wi, ps, dg, width):
                if layer == 1:
                    dn = wpool.tile([P, H], dt.float32, tag="dn")
                    nc.vector.tensor_scalar(
                        out=dn[:],
                        in0=ps[:].rearrange("p (h y) -> p h y", y=dg + 2)[:, :, dg:dg + 1],
                        scalar1=1e-30, scalar2=None, op0=Alu.max)
                    rc = wpool.tile([P, H], dt.float32, tag="rc")
                    nc.vector.reciprocal(out=rc[:], in_=dn[:])
                    hr = wpool.tile([P, CIN], dt.float32, tag="hr")
                    nc.vector.tensor_tensor(
                        out=hr[:].rearrange("p (h d) -> p h d", h=H),
                        in0=ps[:].rearrange("p (h y) -> p h y", y=dg + 2)[:, :, 0:dg],
                        in1=rc[:].unsqueeze(2).to_broadcast([P, H, dg]),
                        op=Alu.mult)
                    nc.vector.tensor_tensor(out=hr[:], in0=hr[:], in1=b1_sb[:],
                                            op=Alu.add)
                    # leaky_relu on the scalar engine, straight into hp_sb
                    nc.scalar.activation(
                        out=hp_sb[:, wi * CIN:(wi + 1) * CIN], in_=hr[:],
                        func=mybir.ActivationFunctionType.Prelu, alpha=NEG)
                    node_l2(wi)
                    if wi == NWA - 1:
                        ag(t2ownA, T2A, ROWS_A)   # overlap AG2A with edge1 tail
                else:
                    dn = wpool.tile([P, H], dt.float32, tag="dn")
                    nc.vector.tensor_scalar(
                        out=dn[:],
                        in0=ps[:].rearrange("p (h y) -> p h y", y=dg + 2)[:, :, dg:dg + 1],
                        scalar1=1e-30, scalar2=4.0, op0=Alu.max,
                        op1=Alu.mult)
                    rc = wpool.tile([P, H], dt.float32, tag="rc")
                    nc.vector.reciprocal(out=rc[:], in_=dn[:])
                    tmp = wpool.tile([P, H * D2], dt.float32, tag="otmp")
                    nc.vector.tensor_tensor(
                        out=tmp[:].rearrange("p (h d) -> p h d", h=H),
                        in0=ps[:].rearrange("p (h y) -> p h y", y=dg + 2)[:, :, 0:dg],
                        in1=rc[:].unsqueeze(2).to_broadcast([P, H, D2]),
                        op=Alu.mult)
                    red = wpool.tile([P, D2], dt.float32, tag="red")
                    nc.vector.tensor_reduce(
                        out=red[:],
                        in_=tmp[:].rearrange("p (h d) -> p d h", h=H),
                        axis=mybir.AxisListType.X, op=Alu.add)
                    nc.vector.tensor_tensor(out=red[:], in0=red[:], in1=b2_sb[:],
                                            op=Alu.add)
                    nc.sync.dma_start(out=out_t[wi * P:(wi + 1) * P, :], in_=red[:])

            # ---------- program ----------
            for wi in range(NWA):
                node_l1(wi)
            ag(t1ownA, T1A, ROWS_A)
            for wi in range(NWA, NW):
                node_l1(wi)
            ag(t1ownB, T1B, ROWS_B)
            edge_phase(1)            # fuses node_l2; AG2A fires after window 24
            ag(t2ownB, T2B, ROWS_B)
            edge_phase(2)

    nc.compile()
    return nc


def _host_inputs(inputs, sched):
    x = np.asarray(inputs['x'], np.float32)
    W1 = np.asarray(inputs['W1'], np.float32)
    W2 = np.asarray(inputs['W2'], np.float32)
    a_src1 = np.asarray(inputs['a_src1'], np.float32)
    a_dst1 = np.asarray(inputs['a_dst1'], np.float32)
    a_src2 = np.asarray(inputs['a_src2'], np.float32)
    a_dst2 = np.asarray(inputs['a_dst2'], np.float32)
    b1 = np.asarray(inputs['b1'], np.float32)
    b2 = np.asarray(inputs['b2'], np.float32)

    a1r = np.tile(np.concatenate([a_src1.reshape(-1), a_dst1.reshape(-1)])[None, :],
                  (P, 1)).astype(np.float32)
    a2r = np.tile(np.concatenate([a_src2.reshape(-1), a_dst2.reshape(-1)])[None, :],
                  (P, 1)).astype(np.float32)
    b1r = np.tile(b1[None, :], (P, 1)).astype(np.float32)
    b2r = np.tile(b2[None, :], (P, 1)).astype(np.float32)
    ident = np.eye(P, dtype=np.float32).astype(bf16)

    in_maps = []
    for k in range(NCORE):
        xk = np.zeros((NSHP, CIN), np.float32)
        xk[:NSH] = x[k * NSH:(k + 1) * NSH]
        in_maps.append({
            "xT": np.ascontiguousarray(xk.T).astype(bf16),
            "W1b": W1.astype(bf16),
            "W2b": W2.astype(bf16),
            "a1r": a1r, "a2r": a2r, "b1r": b1r, "b2r": b2r,
            "ident_in": ident,
            "sidx": sched['srcpk'][k],
            "cnt_in": sched['cnts'][k][None, :],
            "mtin": sched['mt_host'][k],
            "mtTin": sched['mtT_host'][k],
        })
    return in_maps


def kernel(**inputs):
    import os
    from concourse.bass_utils import run_bass_kernel_spmd

    edge_index = np.asarray(inputs['edge_index'])
    sched = _prep(edge_index)
    nc = _build(sched)
    in_maps = _host_inputs(inputs, sched)

    trace = os.environ.get("KERNEL_TRACE") == "1"
    if trace:
        try:
            import profhook
            profhook.install()
        except ImportError:
            pass
    res = run_bass_kernel_spmd(nc, in_maps, core_ids=list(range(NCORE)),
                               trace=trace)
    if trace and res.exec_time_ns:
        print(f"HW exec time: {res.exec_time_ns} ns")
        kernel.exec_time_ns = res.exec_time_ns
        kernel.res = res

    out = np.zeros((N, D2), np.float32)
    for k in range(NCORE):
        out[k * NSH:(k + 1) * NSH] = res.results[k]["out"][:NSH]
    return out
